# revision 41
# baseline (speedup 1.0000x reference)
"""Trainium2 Bass kernel for multi-head attention (B=2, S=2048, D=1024, H=16).

v6: token-parallel with baked weights, AllGather-x + on-core kv recompute.

v5 AllGathered k/v (8 MB out per core at the modeled 40 GB/s collective
bandwidth = ~270 us, the device-side critical path).  v6 instead AllGathers
the raw x shards (4 MB out, and the collective starts at t=0 with no compute
prerequisite) and recomputes k/v for the whole batch on every core (+82 us of
PE time, hidden under the collective):

  - weights are embedded in the NEFF as Const tensors (zero per-exec wire),
    one tile+DMA per matrix in first-use order so q-proj starts early;
  - core c owns tokens [512c:512(c+1)) (cores 0-3 = batch 0, 4-7 = batch 1);
    wire in = its x^T shard (1 MB bf16, rows p-major for long DMA runs),
    wire out = its y^T shard;
  - x is AllGathered within each 4-core batch group in CWS token chunks;
    per chunk every core projects k (feature-major) and v (token-major --
    no PE transposes) for all 4 members' tokens, then runs the attention
    inner loop (scores -> exp -> attn@V + colsum row), software-pipelined
    by one step so the PE never stalls on the Act engine's exp;
  - each core's attention covers all 16 heads for its 512 queries, so the
    out-projection directly yields its exact y^T shard -- no ReduceScatter.

host: y[512c:512(c+1), :] = yo_c^T;  y += b_out
"""

import os
import sys

for _p in ("/opt/trn_rl_repo",):
    if _p not in sys.path and os.path.isdir(_p):
        sys.path.insert(0, _p)

import numpy as np

# Problem shapes (hardcoded per contest rules).
B, S, D, H = 2, 2048, 1024, 16
DH = D // H            # 64
NCORES = 8
GPC = 4                # cores per batch group
BS = B * S             # 4096 tokens
TPC = BS // NCORES     # 512 tokens per core
KT = D // 128          # 8 contraction k-tiles
NHH = H // 2           # 8 head pairs
CWS = (128, 128, 128, 128)  # x AllGather chunk sizes (own tokens per chunk);
                            # all 128-aligned so v-projection token blocks
                            # stay full-width, and each must divide 512


def build_program(w_qkv, w_out, cws=None):
    """Build + compile the per-core Bass program (SPMD, symmetric; weights
    baked as Const tensors)."""
    import concourse.bass as bass
    import concourse.mybir as mybir
    import concourse.tile as tile
    from concourse import bacc
    import ml_dtypes

    d, h, dh, kt = D, H, DH, KT
    cws = tuple(cws) if cws is not None else CWS
    assert sum(cws) == TPC and all(c % 128 == 0 for c in cws)
    mxw = max(cws)                     # tile sizing for the largest chunk
    lq = TPC                           # local queries
    f32 = mybir.dt.float32
    bf16 = mybir.dt.bfloat16
    AF = mybir.ActivationFunctionType
    groups = [[0, 1, 2, 3], [4, 5, 6, 7]]

    nc = bacc.Bacc("TRN2", target_bir_lowering=False, debug=False,
                   num_devices=NCORES, enable_partition_id=False)

    # wire: x^T shard, rows p-major (p, k) so every DMA run (stage, xloc,
    # gathered loads) is a contiguous k*m block instead of 256-byte slivers
    xs_d = nc.dram_tensor("xs", [d, lq], bf16, kind="ExternalInput")
    yo_d = nc.dram_tensor("yo", [d, lq], bf16, kind="ExternalOutput")

    # baked weights: SBUF layout [128, kt, P, 1024]; P in (q, k, v, o)
    Wq = np.asarray(w_qkv[:, 0:d], np.float32)
    Wk = np.asarray(w_qkv[:, d:2 * d], np.float32)
    Wv = np.asarray(w_qkv[:, 2 * d:3 * d], np.float32)
    Wo = np.asarray(w_out, np.float32)
    wall_np = (np.stack([Wq, Wk, Wv, Wo], 0)          # (P, d, c)
               .reshape(4, kt, 128, d)                # (P, k, p, c)
               .transpose(2, 1, 0, 3)                 # (p, k, P, c)
               .reshape(128, kt * 4 * d)).astype(ml_dtypes.bfloat16)
    wall_d = nc.inline_tensor(np.ascontiguousarray(wall_np), name="wall")

    # AG bounce (collectives cannot read IO tensors directly) + gathered x
    # per chunk (gathered rows member-major (j, k, p))
    xin = [nc.dram_tensor(f"xin{c}", [d, cw], bf16, kind="Internal")
           for c, cw in enumerate(cws)]
    xg = [nc.dram_tensor(f"xg{c}", [GPC * d, cw], bf16, kind="Internal")
          for c, cw in enumerate(cws)]

    with tile.TileContext(nc) as tc:
        with (
            tc.tile_pool(name="p_w", bufs=1) as p_w,
            tc.tile_pool(name="p_x", bufs=1) as p_x,
            tc.tile_pool(name="p_xg", bufs=1) as p_xg,
            tc.tile_pool(name="p_kk", bufs=2) as p_kk,
            tc.tile_pool(name="p_v", bufs=2) as p_v,
            tc.tile_pool(name="p_exp", bufs=3) as p_exp,
            tc.tile_pool(name="p_ao", bufs=1) as p_ao,
            tc.tile_pool(name="p_y", bufs=1) as p_y,
            tc.tile_pool(name="p_misc", bufs=1) as p_misc,
            tc.tile_pool(name="pp_mm", bufs=2, space="PSUM") as pp_mm,
            tc.tile_pool(name="pp_sc", bufs=2, space="PSUM") as pp_sc,
            tc.tile_pool(name="pp_o", bufs=1, space="PSUM") as pp_o,
        ):
            # ---- x AllGather first: zero compute prerequisite ----
            off = 0
            for t, cw in enumerate(cws):
                nc.sync.dma_start(xin[t].ap(), xs_d.ap()[:, off:off + cw])
                nc.gpsimd.collective_compute(
                    "AllGather", mybir.AluOpType.bypass, replica_groups=groups,
                    ins=[xin[t].ap()], outs=[xg[t].ap()])
                off += cw

            # one tile+DMA per weight matrix, in first-use order (q, k, v, o),
            # so the q projection starts ~10us in instead of behind all 8 MB
            wsb = []
            for P in (0, 1, 2, 3):
                wt = p_w.tile([128, kt, d], bf16, tag=f"w{P}", name=f"w{P}")
                nc.gpsimd.dma_start(
                    wt[:],
                    wall_d.ap().rearrange("p (k P c) -> p k P c", k=kt,
                                          P=4)[:, :, P, :])
                wsb.append(wt)

            def w_view(P, k, nb):
                return wsb[P][:, k, nb * 128:nb * 128 + 128]

            def w_row(P, k, half):
                return wsb[P][:, k, half * 512:half * 512 + 512]

            xloc = p_x.tile([128, kt, lq], bf16)
            nc.sync.dma_start(
                xloc[:], xs_d.ap().rearrange("(p k) m -> p k m", p=128))

            # ---- q projection (overlaps AG chunk 0) ----
            qT_sb = p_x.tile([128, kt, lq], bf16)
            for nb in range(kt):
                pm = pp_mm.tile([128, 512], f32, tag="mm", name="pq")
                for k in range(kt):
                    nc.tensor.matmul(pm[:, :lq], w_view(0, k, nb),
                                     xloc[:, k, :], start=(k == 0),
                                     stop=(k == kt - 1))
                nc.vector.tensor_copy(qT_sb[:, nb, :], pm[:, :lq])

            # ---- attention: accumulate attn@V + colsum per head pair ----
            # aoacc[hh]: [65, lq]; rows 0-63 attn@V, row 64 colsum; one pair
            # (A, B) per head pair; accumulated over chunks in bf16 SBUF.
            aoacc = [(p_ao.tile([65, lq], bf16, tag=f"aoa{hh}", name=f"aoa{hh}"),
                      p_ao.tile([65, lq], bf16, tag=f"aob{hh}", name=f"aob{hh}"))
                     for hh in range(NHH)]
            aoT_sb = p_ao.tile([128, kt, lq], bf16, tag="aoT", name="aoT")

            def emit_norm(hh):
                # normalize head pair hh -> aoT; emitted right after hh's
                # final accumulate so it overlaps the remaining head pairs'
                # attention instead of queueing after all of them
                aoa, aob = aoacc[hh]
                rca = p_misc.tile([1, lq], f32, tag="rca", name="rca")
                rcb = p_misc.tile([1, lq], f32, tag="rcb", name="rcb")
                nc.vector.reciprocal(rca[:], aoa[64:65, :])
                nc.vector.reciprocal(rcb[:], aob[64:65, :])
                bca = p_misc.tile([64, lq], f32, tag="bca", name="bca")
                bcb = p_misc.tile([64, lq], f32, tag="bcb", name="bcb")
                nc.gpsimd.partition_broadcast(bca[:], rca[:])
                nc.gpsimd.partition_broadcast(bcb[:], rcb[:])
                nc.vector.tensor_mul(aoT_sb[0:64, hh, :], aoa[0:64, :], bca[:])
                nc.vector.tensor_mul(aoT_sb[64:128, hh, :], aob[0:64, :],
                                     bcb[:])

            for t, cw in enumerate(cws):
                gw = GPC * cw          # gathered tokens this chunk
                nkb = gw // 128        # 128-key blocks this chunk
                tbn = cw // 128        # 128-token blocks per member
                xgt = p_xg.tile([128, GPC, kt, mxw], bf16, tag="xgt",
                                name=f"xgt{t}")
                for j in range(GPC):
                    nc.sync.dma_start(
                        xgt[:, j, :, :cw],
                        xg[t].ap()[j * d:(j + 1) * d, :]
                        .rearrange("(p k) m -> p k m", p=128))

                # k projection (feature-major): kk[:, nb, :] over gathered
                # tokens, in 512-token groups (one PSUM bank per group)
                kk = p_kk.tile([128, kt, GPC * mxw], bf16, tag="kk",
                               name=f"kk{t}")
                jg = 512 // cw         # members per 512-token group
                for nb in range(kt):
                    for g in range(gw // 512):
                        pm = pp_mm.tile([128, 512], f32, tag="mm", name="pk")
                        for k in range(kt):
                            nc.tensor.matmul(
                                pm[:, :], w_view(1, k, nb),
                                xgt[:, jg * g:jg * (g + 1), k, :cw],
                                start=(k == 0), stop=(k == kt - 1))
                        nc.vector.tensor_copy(kk[:, nb, 512 * g:512 * (g + 1)],
                                              pm[:, :])

                # v projection (token-major; vf halves): vex[tok, kb, hh, 130]
                vex = p_v.tile([128, GPC * mxw // 128, NHH, 130], bf16,
                               tag="vex", name=f"vex{t}")
                nc.vector.memset(vex[:, :, :, 64:65], 1.0)
                nc.vector.memset(vex[:, :, :, 129:130], 1.0)
                for j in range(GPC):
                    for tb in range(tbn):
                        kb = j * tbn + tb
                        for half in range(2):
                            pv = pp_mm.tile([128, 4, 2, 64], f32, tag="mm",
                                            name="pv")
                            for k in range(kt):
                                nc.tensor.matmul(
                                    pv[:, :, :, :],
                                    xgt[:, j, k, 128 * tb:128 * (tb + 1)],
                                    w_row(2, k, half),
                                    start=(k == 0), stop=(k == kt - 1))
                            hb = 4 * half
                            nc.vector.tensor_copy(vex[:, kb, hb:hb + 4, 0:64],
                                                  pv[:, :, 0, :])
                            nc.vector.tensor_copy(vex[:, kb, hb:hb + 4, 65:129],
                                                  pv[:, :, 1, :])

                # scores -> exp -> attn@V (+colsum), accumulating over kb.
                # Software-pipelined by one step: attn@V of step s is emitted
                # after the scores of step s+1, so the PE's in-order queue
                # never stalls waiting for step s's exp on the Act engine.
                po = {}

                def emit_attnv(hh, kb, ex):
                    if kb == 0:
                        # allocated here (not at the scores step) so the
                        # bufs=1 slot rotation happens after every pending
                        # write to the previous head pair's tiles is emitted
                        po[hh] = (pp_o.tile([65, lq], f32, tag="poa",
                                            name="poa"),
                                  pp_o.tile([65, lq], f32, tag="pob",
                                            name="pob"))
                    poa, pob = po[hh]
                    nc.tensor.matmul(poa[:, :], vex[:, kb, hh, 0:65],
                                     ex[:, 0:lq], start=(kb == 0),
                                     stop=(kb == nkb - 1))
                    nc.tensor.matmul(pob[:, :], vex[:, kb, hh, 65:130],
                                     ex[:, 512:512 + lq], start=(kb == 0),
                                     stop=(kb == nkb - 1))
                    if kb == nkb - 1:
                        poa, pob = po.pop(hh)
                        aoa, aob = aoacc[hh]
                        if t == 0:
                            nc.vector.tensor_copy(aoa[:], poa[:])
                            nc.vector.tensor_copy(aob[:], pob[:])
                        else:
                            nc.vector.tensor_add(aoa[:], aoa[:], poa[:])
                            nc.vector.tensor_add(aob[:], aob[:], pob[:])
                        if t == len(cws) - 1:
                            emit_norm(hh)

                prev = None
                for hh in range(NHH):
                    for kb in range(nkb):
                        ks = 128 * kb
                        psc = pp_sc.tile([128, 1024], f32, tag="sc", name="psc")
                        nc.tensor.matmul(psc[:, 0:lq],
                                         kk[0:64, hh, ks:ks + 128],
                                         qT_sb[0:64, hh, :], start=True,
                                         stop=True, tile_position=(0, 0))
                        nc.tensor.matmul(psc[:, 512:512 + lq],
                                         kk[64:128, hh, ks:ks + 128],
                                         qT_sb[64:128, hh, :], start=True,
                                         stop=True, tile_position=(64, 0))
                        ex = p_exp.tile([128, 1024], bf16, tag="exp", name="ex")
                        nc.scalar.activation(ex[:], psc[:], AF.Exp, scale=0.125)
                        if prev is not None:
                            emit_attnv(*prev)
                        prev = (hh, kb, ex)
                emit_attnv(*prev)

            # ---- out-projection (exact y^T shard), output streamed per
            # feature block so the last DMA overlaps earlier copies ----
            yst = p_y.tile([128, kt, lq], bf16, tag="yst", name="yst")
            yo_v = yo_d.ap().rearrange("(n p) m -> p n m", p=128)
            for nb in range(kt):
                py = pp_mm.tile([128, 512], f32, tag="mm", name="py")
                for k in range(kt):
                    nc.tensor.matmul(py[:, :lq], w_view(3, k, nb),
                                     aoT_sb[:, k, :], start=(k == 0),
                                     stop=(k == kt - 1))
                nc.vector.tensor_copy(yst[:, nb, :], py[:, :lq])
                nc.sync.dma_start(yo_v[:, nb, :], yst[:, nb, :])

    nc.compile()
    return nc


_CACHE = {}


def _bf16():
    import ml_dtypes
    return ml_dtypes.bfloat16


def _prep_inputs(x):
    """Host-side shard prep: per-core x^T token shards (bf16 wire), rows
    permuted (k, p) -> (p, k) to give on-device DMAs long contiguous runs."""
    bf16 = _bf16()
    b, s, d = x.shape
    kt = KT
    xT = np.ascontiguousarray(x.reshape(BS, d).T).astype(bf16)   # [d, bs]
    out = []
    for c in range(NCORES):
        sh = xT[:, TPC * c:TPC * (c + 1)]                        # [(k p), m]
        sh = sh.reshape(kt, 128, TPC).transpose(1, 0, 2)         # (p, k, m)
        out.append({"xs": np.ascontiguousarray(sh.reshape(d, TPC))})
    return out


class _PjrtRunner:
    """Caches the shard_map-jitted executable for a compiled Bass program so it
    can be invoked (and timed) repeatedly."""

    def __init__(self, nc, n_cores=NCORES):
        import jax
        import numpy as _np
        import concourse.mybir as mybir
        from concourse import bass2jax
        from jax.sharding import Mesh, PartitionSpec
        from jax.experimental.shard_map import shard_map

        bass2jax.install_neuronx_cc_hook()
        self.jax = jax
        self.nc = nc
        self.n_cores = n_cores
        partition_name = (nc.partition_id_tensor.name
                          if nc.partition_id_tensor else None)
        self.partition_name = partition_name
        in_names, out_names, out_avals, zero_outs = [], [], [], []
        for alloc in nc.m.functions[0].allocations:
            if not isinstance(alloc, mybir.MemoryLocationSet):
                continue
            if alloc.kind not in ("ExternalInput", "ExternalOutput"):
                continue
            name = alloc.memorylocations[0].name
            if alloc.kind == "ExternalInput":
                if name != partition_name:
                    in_names.append(name)
            elif alloc.kind == "ExternalOutput":
                out_names.append(name)
                shape = tuple(alloc.tensor_shape)
                dtype = mybir.dt.np(alloc.dtype)
                out_avals.append(jax.core.ShapedArray(shape, dtype))
                zero_outs.append(_np.zeros(shape, dtype))
        self.in_names, self.out_names = in_names, out_names
        self.out_avals, self.zero_outs = out_avals, zero_outs
        n_params, n_outs = len(in_names), len(out_names)
        self.n_params, self.n_outs = n_params, n_outs
        # The neuron lowering's custom call takes ONLY the ExternalInput
        # operands; NKI allocates the outputs itself.  Passing zero output
        # buffers (the historical pattern) marshals dead operands through
        # the axon tunnel every exec -- so we don't.
        all_names = list(in_names)
        if partition_name is not None:
            all_names = all_names + [partition_name]

        def _body(*args):
            operands = list(args)
            if partition_name is not None:
                operands.append(bass2jax.partition_id_tensor())
            outs = bass2jax._bass_exec_p.bind(
                *operands,
                out_avals=tuple(out_avals),
                in_names=tuple(all_names),
                out_names=tuple(out_names),
                lowering_input_output_aliases=(),
                sim_require_finite=True,
                sim_require_nnan=True,
                nc=nc,
            )
            return tuple(outs)

        self._body = _body
        devices = jax.devices()[:n_cores]
        assert len(devices) == n_cores
        mesh = Mesh(np.asarray(devices), ("core",))
        in_specs = (PartitionSpec("core"),) * n_params
        out_specs = (PartitionSpec("core"),) * n_outs
        self.fn = jax.jit(
            shard_map(_body, mesh=mesh, in_specs=in_specs, out_specs=out_specs,
                      check_rep=False),
            keep_unused=True,
        )
        self.mesh = mesh
        self._dev_inputs = None

    def set_inputs(self, in_maps):
        import jax
        concat_in = [
            np.concatenate([np.asarray(in_maps[c][n]) for c in range(self.n_cores)],
                           axis=0)
            for n in self.in_names
        ]
        self._dev_inputs = [jax.device_put(a) for a in concat_in]

    def run(self):
        out_arrs = self.fn(*self._dev_inputs)
        out_arrs = [np.asarray(o) for o in out_arrs]
        return [
            {n: out_arrs[i].reshape(self.n_cores, *self.out_avals[i].shape)[c]
             for i, n in enumerate(self.out_names)}
            for c in range(self.n_cores)
        ]

    def _timing_fn(self):
        return self.fn

    def time_exec(self, iters=10, burst=16):
        """Per-exec time via async burst differencing.

        Alternates (small, big) burst rounds and differences the MIN wall
        time of each length: rig load only ever inflates a wall-clock
        sample, so min-per-length converges to the quiet-window time and
        the fixed dispatch term cancels in the difference.  Transient axon
        failures (mesh desync) are retried.
        """
        import time
        fn = self._timing_fn()
        out = fn(*self._dev_inputs)
        self.jax.block_until_ready(out)

        def run_burst(n):
            t0 = time.perf_counter()
            outs = None
            for _ in range(n):
                outs = fn(*self._dev_inputs)
            self.jax.block_until_ready(outs)
            return time.perf_counter() - t0

        small = max(2, burst // 4)
        singles, smalls, bursts = [], [], []
        rounds = max(6, iters)
        for r in range(rounds):
            try:
                singles.append(run_burst(1))
                smalls.append(run_burst(small))
                bursts.append(run_burst(burst))
            except Exception:
                time.sleep(2)
                continue
            if r + 1 < rounds:
                # spread rounds across wall-clock so the per-length minima
                # sample several external-load windows, not one burst of them
                time.sleep(0.25)
        if not smalls or not bursts:
            raise RuntimeError("time_exec: all burst rounds failed")
        singles.sort()
        smalls.sort()
        bursts.sort()
        per_exec = (bursts[0] - smalls[0]) / (burst - small)
        return per_exec, {"single": singles, "small": smalls, "burst": bursts,
                          "burst_n": burst, "small_n": small}


def _get_runner(w_qkv=None, w_out=None):
    if w_qkv is None:
        return _CACHE["runner"]
    key = (hash(np.asarray(w_qkv, np.float32).tobytes()),
           hash(np.asarray(w_out, np.float32).tobytes()))
    if _CACHE.get("key") != key or "runner" not in _CACHE:
        nc = build_program(np.asarray(w_qkv, np.float32),
                           np.asarray(w_out, np.float32))
        _CACHE["nc"] = nc
        _CACHE["runner"] = _PjrtRunner(nc)
        _CACHE["key"] = key
    return _CACHE["runner"]


def run_on_hw(x, w_qkv, w_out, b_out, trace=False):
    results = None
    for attempt in range(2):
        try:
            r = _get_runner(w_qkv, w_out)
            r.set_inputs(_prep_inputs(np.asarray(x)))
            results = r.run()
            break
        except Exception:
            if attempt == 1:
                raise
            # transient NRT exec-unit wedge: rebuild the backend + runner once
            _CACHE.clear()
            import time as _time
            try:
                import jax
                jax.clear_caches()
            except Exception:
                pass
            _time.sleep(2)
    y = np.empty((BS, D), dtype=np.float32)
    for c in range(NCORES):
        y[TPC * c:TPC * (c + 1), :] = results[c]["yo"].astype(np.float32).T
    y = y.reshape(B, S, D) + np.asarray(b_out, np.float32)[None, None, :]
    return y.astype(np.float32), results


def kernel(**inputs):
    y, _ = run_on_hw(inputs["x"], inputs["w_qkv"], inputs["w_out"], inputs["b_out"])
    return y
